# revision 1
# baseline (speedup 1.0000x reference)
"""Trainium2 Bass kernel for nn_MultiHeadNetwork (moe_routing).

Strategy
--------
Host side (numpy, inside kernel()):
  * task id per row = argmax of the trailing one-hot block of x (data, not
    activation dependent), rows sorted by task id, batch split into 8
    contiguous 512-row chunks (one per NeuronCore).
  * Trunk weights replicated; per core the head only needs the few tasks its
    chunk spans.  The 512 columns are split into two 256-column windows; for
    each window the spanned tasks become "slots" (padded to the max SW over
    all cores/windows so the SPMD program structure is uniform; the slot
    *weights and masks* are per-core data).
  * Everything is pre-rounded to the fp32r grid (11 mantissa bits) and packed
    so every DMA is a contiguous [128, F] panel.

Device side (one SPMD Tile program on 8 cores):
  * Activations kept feature-major (hT: [feat partitions, batch free]) so each
    trunk layer is out = W_chunk.T @ hT with NO transposes anywhere.
  * All matmuls float32r (fp32 storage, full PE rate at N >= 256).
  * Trunk layers run k-OUTER over half-width (8 w-chunks -> 8 PSUM banks):
    the first matmul only needs one k-tile of activations + one weight panel,
    so the PE starts almost immediately and layer transitions don't stall.
  * ReLU + bias fused on the scalar engine straight out of PSUM.
  * Head: per (window, slot): psum = head_W[slot].T @ h3T[window cols], then
    copy_predicated with a host 0/1 mask selects the rows of that task.
  * Head bias and the inverse permutation are applied on host.
"""

import numpy as np
from contextlib import ExitStack

import concourse.bacc as bacc
import concourse.mybir as mybir
from concourse.tile import TileContext
from concourse import bass_utils

BATCH = 4096
FEAT = 2048
NUM_TASKS = 50
WIDTH = 2048
HEAD_DIM = 256
NCORES = 8
BPC = BATCH // NCORES          # 512 rows per core
NWIN = 2                       # head column windows per core
WINC = BPC // NWIN             # 256 columns per window
KIN = FEAT + NUM_TASKS         # 2098
KC0 = 17                       # input K chunks (zero-padded to 2176)
KPAD = KC0 * 128
KC = WIDTH // 128              # 16
WC = WIDTH // 128              # 16
HALF = WC // 2                 # 8 w-chunks per half-layer
QW = 4                         # w-chunks per trunk quarter (4 PSUM banks)
NQ = WC // QW                  # 4 quarters per layer
MH = HEAD_DIM // 128           # 2 head-dim halves

F32 = mybir.dt.float32
F32R = mybir.dt.float32r
U8 = mybir.dt.uint8

_PROG_CACHE: dict = {}


def round_fp32r(a: np.ndarray) -> np.ndarray:
    """Round fp32 to the fp32r grid (11 mantissa bits, RNE) like the HW does."""
    b = np.ascontiguousarray(a, np.float32).view(np.uint32)
    bias = np.uint32(0x7FF) + ((b >> np.uint32(12)) & np.uint32(1))
    out = (b + bias) & np.uint32(0xFFFFF000)
    return out.view(np.float32)


def _build(S: int, repeat: int = 1):
    """Build + compile the SPMD Tile program.  S = head slots per window.

    repeat > 1 wraps the whole body in a hardware For_i loop (benchmarking
    only: amortizes launch/RPC overhead across repeat executions).
    """
    nslot = NWIN * S
    nc = bacc.Bacc("TRN2", target_bir_lowering=False, debug=False)
    xT = nc.dram_tensor("xT", [KC0, 128, BPC], F32R, kind="ExternalInput").ap()
    w0 = nc.dram_tensor("w0p", [NQ, KC0, 128, QW * 128], F32R, kind="ExternalInput").ap()
    w1 = nc.dram_tensor("w1p", [NQ, KC, 128, QW * 128], F32R, kind="ExternalInput").ap()
    w2 = nc.dram_tensor("w2p", [NQ, KC, 128, QW * 128], F32R, kind="ExternalInput").ap()
    bia = nc.dram_tensor("bias", [128, 3 * WC], F32, kind="ExternalInput").ap()
    hws = nc.dram_tensor("hws", [nslot, 128, KC * HEAD_DIM], F32R, kind="ExternalInput").ap()
    msk = nc.dram_tensor("msk", [128, nslot * WINC], U8, kind="ExternalInput").ap()
    out = nc.dram_tensor("outT", [MH, 128, BPC], F32, kind="ExternalOutput").ap()

    with TileContext(nc) as tc, ExitStack() as ctx:
        # xT (17 tiles) and h2 (16) share slots: h2 allocates only after
        # layer 0 fully finished reading xT.  h3 gets its own pool (h1 is
        # still being read while h3 is produced).
        actA = ctx.enter_context(tc.tile_pool(name="actA", bufs=KC0))
        actB = ctx.enter_context(tc.tile_pool(name="actB", bufs=KC))
        wp = ctx.enter_context(tc.tile_pool(name="wp", bufs=12))
        cons = ctx.enter_context(tc.tile_pool(name="cons", bufs=1))
        hwp = ctx.enter_context(tc.tile_pool(name="hwp", bufs=5))
        op = ctx.enter_context(tc.tile_pool(name="op", bufs=MH))
        psp = ctx.enter_context(tc.tile_pool(name="psp", bufs=8, space="PSUM"))

        if repeat > 1:
            ctx.enter_context(tc.For_i(0, repeat, 1))

        bt = cons.tile([128, 3 * WC], F32, tag="bt")
        nc.sync.dma_start(bt[:], bia)
        mt = cons.tile([128, nslot * WINC], U8, tag="mt")
        nc.sync.dma_start(mt[:], msk)

        xt = [None] * KC0

        def trunk_layer(src, wdram, nk, li, pool, tag, load_x=False):
            outs = [None] * WC
            for q in range(NQ):
                pss = [
                    psp.tile([128, BPC], F32, tag="ps", name=f"psL{li}q{q}w{w}")
                    for w in range(QW)
                ]
                for k in range(nk):
                    wt = wp.tile([128, QW * 128], F32R, tag="wp", name=f"wtL{li}q{q}k{k}")
                    nc.sync.dma_start(wt[:], wdram[q, k])
                    if load_x and q == 0:
                        t = actA.tile([128, BPC], F32R, tag="actA", name=f"xt{k}")
                        nc.sync.dma_start(t[:], xT[k])
                        src[k] = t
                    for w in range(QW):
                        nc.tensor.matmul(
                            pss[w][:],
                            wt[:, w * 128:(w + 1) * 128],
                            src[k][:],
                            start=(k == 0),
                            stop=(k == nk - 1),
                        )
                for w in range(QW):
                    wc_i = q * QW + w
                    h = pool.tile([128, BPC], F32R, tag=tag, name=f"h{li}_{wc_i}")
                    nc.scalar.activation(
                        h[:], pss[w][:], mybir.ActivationFunctionType.Relu,
                        bias=bt[:, li * WC + wc_i: li * WC + wc_i + 1],
                    )
                    outs[wc_i] = h
            return outs

        h1 = trunk_layer(xt, w0, KC0, 0, actB, "actB", load_x=True)
        h2 = trunk_layer(h1, w1, KC, 1, actA, "actA")
        h3 = trunk_layer(h2, w2, KC, 2, actB, "actB")

        om = [op.tile([128, BPC], F32, tag="op", name=f"om{m}") for m in range(MH)]
        for win in range(NWIN):
            cols = slice(win * WINC, (win + 1) * WINC)
            for s in range(S):
                sl = win * S + s
                hw = hwp.tile([128, KC * HEAD_DIM], F32R, tag="hwp", name=f"hw{sl}")
                nc.sync.dma_start(hw[:], hws[sl])
                for m in range(MH):
                    ps = psp.tile([128, WINC], F32, tag="ps", name=f"psH{sl}m{m}")
                    for k in range(KC):
                        nc.tensor.matmul(
                            ps[:],
                            hw[:, k * HEAD_DIM + m * 128: k * HEAD_DIM + (m + 1) * 128],
                            h3[k][:, cols],
                            start=(k == 0),
                            stop=(k == KC - 1),
                        )
                    if s == 0:
                        nc.vector.tensor_copy(om[m][:, cols], ps[:])
                    else:
                        nc.vector.copy_predicated(
                            om[m][:, cols], mt[:, sl * WINC:(sl + 1) * WINC], ps[:]
                        )
            for m in range(MH):
                nc.sync.dma_start(out[m][:, cols], om[m][:, cols])

    nc.compile()
    return nc


def _pack_w(W, nk):
    # [NQ, nk, 128, QW*128]; [q, k, kp, w*128+m] = W[k*128+kp, (q*QW+w)*128+m]
    return np.ascontiguousarray(
        W.reshape(nk, 128, NQ, QW * 128).transpose(2, 0, 1, 3)
    )


def _pack_trunk(W0, W1, W2, b0, b1, b2):
    W0pad = np.zeros((KPAD, WIDTH), np.float32)
    W0pad[:KIN] = round_fp32r(W0)
    w0p = _pack_w(W0pad, KC0)
    w1p = _pack_w(round_fp32r(W1), KC)
    w2p = _pack_w(round_fp32r(W2), KC)
    bias = np.zeros((128, 3 * WC), np.float32)
    for li, b in enumerate((b0, b1, b2)):
        bias[:, li * WC:(li + 1) * WC] = b.reshape(WC, 128).T
    return w0p, w1p, w2p, bias


def prepare(x, W0, b0, W1, b1, W2, b2, head_W, head_b):
    """Host-side sharding. Returns (in_maps, order, sorted_task_ids, S)."""
    x = np.asarray(x, np.float32)
    W0 = np.asarray(W0, np.float32)
    W1 = np.asarray(W1, np.float32)
    W2 = np.asarray(W2, np.float32)
    b0 = np.asarray(b0, np.float32)
    b1 = np.asarray(b1, np.float32)
    b2 = np.asarray(b2, np.float32)
    head_W = np.asarray(head_W, np.float32)

    tid = np.argmax(x[:, -NUM_TASKS:], axis=1)
    order = np.argsort(tid, kind="stable")
    x_s = x[order]
    t_s = tid[order]

    # per (core, window) spanned task lists
    win_tasks = []   # [core][win] -> list of tasks
    for c in range(NCORES):
        per_win = []
        for w in range(NWIN):
            lo = c * BPC + w * WINC
            ch = t_s[lo: lo + WINC]
            per_win.append(list(dict.fromkeys(ch.tolist())))
        win_tasks.append(per_win)
    S = max(len(tl) for per in win_tasks for tl in per)

    w0p, w1p, w2p, bias = _pack_trunk(W0, W1, W2, b0, b1, b2)
    head_W = round_fp32r(head_W)
    # hw_pack[t, kp, kc*256 + j] = head_W[t, kc*128 + kp, j]
    hw_pack = np.ascontiguousarray(
        head_W.reshape(NUM_TASKS, KC, 128, HEAD_DIM)
        .transpose(0, 2, 1, 3)
        .reshape(NUM_TASKS, 128, KC * HEAD_DIM)
    )

    nslot = NWIN * S
    in_maps = []
    for c in range(NCORES):
        xs = x_s[c * BPC:(c + 1) * BPC]
        xTp = np.zeros((KPAD, BPC), np.float32)
        xTp[:KIN] = round_fp32r(xs.T)
        slot_tasks = []
        msk_c = np.zeros((128, nslot * WINC), np.uint8)
        for w in range(NWIN):
            tl = win_tasks[c][w]
            tl_p = tl + [tl[-1]] * (S - len(tl))
            lo = c * BPC + w * WINC
            ch = t_s[lo: lo + WINC]
            for s, t in enumerate(tl_p):
                sl = w * S + s
                slot_tasks.append(t)
                if 0 < s < len(tl):
                    msk_c[:, sl * WINC:(sl + 1) * WINC] = (ch == t)[None, :].astype(np.uint8)
        hws_c = np.ascontiguousarray(hw_pack[np.asarray(slot_tasks)])
        in_maps.append({
            "xT": np.ascontiguousarray(xTp.reshape(KC0, 128, BPC)),
            "w0p": w0p, "w1p": w1p, "w2p": w2p, "bias": bias,
            "hws": hws_c, "msk": msk_c,
        })
    return in_maps, order, t_s, S


def _assemble(results, order, t_s, head_b):
    head_b = np.asarray(head_b, np.float32)
    outs = []
    for c in range(NCORES):
        oT = results[c]["outT"]                       # [MH, 128, BPC]
        outs.append(oT.reshape(HEAD_DIM, BPC).T)      # [BPC, 256]
    out_s = np.concatenate(outs, axis=0) + head_b[t_s]
    out = np.empty_like(out_s)
    out[order] = out_s
    return out.astype(np.float32)


def kernel(x, W0, b0, W1, b1, W2, b2, head_W, head_b):
    in_maps, order, t_s, S = prepare(x, W0, b0, W1, b1, W2, b2, head_W, head_b)
    nc = _PROG_CACHE.get(S)
    if nc is None:
        nc = _build(S)
        _PROG_CACHE[S] = nc
    res = bass_utils.run_bass_kernel_spmd(nc, in_maps, core_ids=list(range(NCORES)))
    return _assemble(res.results, order, t_s, head_b)



# revision 25
# speedup vs baseline: 1.9785x; 1.9785x over previous
"""Trainium2 Bass kernel for nn_MultiHeadNetwork (moe_routing).

Strategy
--------
Host side (numpy, inside kernel()):
  * task id per row = argmax of the trailing one-hot block of x (data, not
    activation dependent).  Rows are repacked into 32 blocks of 128 rows
    (8 cores x 4 head windows) by a greedy bin-packer that minimizes the
    number of distinct tasks per block (main piece of the largest remaining
    task + exact-subset / best-fit fillers; fragments of a task may be
    non-adjacent).  Blocks are dealt to (core, window) positions heaviest
    window first; the compiled program's per-window slot counts ("struct",
    e.g. (4,2,2,2)) are the max over cores per position.  The inverse row
    permutation is applied to the final output on host.
  * Trunk weights replicated per core; the head only loads the few tasks a
    core's windows span.  Slot weights and masks are per-core data; padded
    slots carry an all-zero mask.
  * All matmul operands are bf16 (fp32 PSUM accumulation); biases and the
    final output stay fp32.  bf16 halves HBM traffic vs fp32/fp32r and runs
    at the same PE rate (1 col/cycle at free dim >= 256; always for bf16).

Device side (one SPMD Tile program on 8 cores, cached per struct):
  * Activations kept feature-major (hT: [feat partitions, batch free]) so each
    trunk layer is out = W_chunk.T @ hT with NO transposes anywhere.
  * Trunk layers run k-OUTER over quarters (4 w-chunks -> 4 PSUM banks):
    the first matmul only needs one k-tile of activations + one weight panel,
    so the PE starts almost immediately and layer transitions don't stall.
  * Weight/x DMAs are grouped 4 k-chunks per transfer (flat per-quarter DRAM
    layout) to keep the serialized HWDGE descriptor-gen off the critical
    path; the very first L0 groups are small ([1,2,4,...]) so the PE starts
    ~2.5us in.  Bias loads after L0-q0's operands; the mask loads in L2.
  * ReLU + bias fused on the scalar engine straight out of PSUM, writing
    bf16 activations.
  * Head (row-major): per 128-row window, stationary = h3 column block
    (shared by all slots per k-chunk), moving = per-slot head weights;
    psum/out = [window rows, HEAD_DIM].  copy_predicated with a host 0/1
    row mask selects each task's rows into the window output, which DMAs
    straight to DRAM in batch-row-major order.  Head weights for all slots
    prefetch during layer 2.
  * Head bias and the inverse permutation are applied on host.

TimelineSim: 194.6us vs 238.2us for the fp32r baseline (HW-measured
324.8us); PE-busy floor is ~184us (trunk 167us + head ~17us).
"""

import numpy as np
import ml_dtypes
from contextlib import ExitStack

import concourse.bacc as bacc
import concourse.mybir as mybir
from concourse.tile import TileContext
from concourse import bass_utils

BATCH = 4096
FEAT = 2048
NUM_TASKS = 50
WIDTH = 2048
HEAD_DIM = 256
NCORES = 8
BPC = BATCH // NCORES          # 512 rows per core
BLK = 128                      # head column window width
NBLK = BPC // BLK              # 4 windows per core
KIN = FEAT + NUM_TASKS         # 2098
KC0 = 17                       # input K chunks (zero-padded to 2176)
KPAD = KC0 * 128
KC = WIDTH // 128              # 16
WC = WIDTH // 128              # 16
QW = 4                         # w-chunks per trunk quarter (4 PSUM banks)
NQ = WC // QW                  # 4 quarters per layer
MH = HEAD_DIM // 128           # 2 head-dim halves

F32 = mybir.dt.float32
BF16 = mybir.dt.bfloat16
U8 = mybir.dt.uint8
NP_BF16 = ml_dtypes.bfloat16

_PROG_CACHE: dict = {}


def _groups(nk, first_small):
    """k-chunk DMA grouping: [(start, len), ...] covering range(nk).
    first_small paces the very first loads so the PE can start early."""
    lens = [1, 2] if first_small else []
    left = nk - sum(lens)
    while left > 0:
        take = min(4, left)
        lens.append(take)
        left -= take
    out, o = [], 0
    for ln in lens:
        out.append((o, ln))
        o += ln
    return out


def _build(struct: tuple, repeat: int = 1):
    """Build + compile the SPMD Tile program.

    struct = per-window head slot counts (len NBLK, descending), e.g.
    (3, 3, 3, 2).  repeat > 1 wraps the body in a hardware For_i loop
    (benchmarking only).
    """
    nslot = sum(struct)
    WCOLS = QW * 128
    nc = bacc.Bacc("TRN2", target_bir_lowering=False, debug=False)
    xT = nc.dram_tensor("xT", [128, KC0 * BPC], BF16, kind="ExternalInput").ap()
    w0 = nc.dram_tensor("w0p", [NQ, 128, KC0 * WCOLS], BF16, kind="ExternalInput").ap()
    w1 = nc.dram_tensor("w1p", [NQ, 128, KC * WCOLS], BF16, kind="ExternalInput").ap()
    w2 = nc.dram_tensor("w2p", [NQ, 128, KC * WCOLS], BF16, kind="ExternalInput").ap()
    bia = nc.dram_tensor("bias", [128, 3 * WC], F32, kind="ExternalInput").ap()
    hws = nc.dram_tensor("hws", [nslot, 128, KC * HEAD_DIM], BF16, kind="ExternalInput").ap()
    msk = nc.dram_tensor("msk", [128, nslot * HEAD_DIM], U8, kind="ExternalInput").ap()
    out = nc.dram_tensor("outT", [NBLK, 128, HEAD_DIM], F32, kind="ExternalOutput").ap()

    with TileContext(nc) as tc, ExitStack() as ctx:
        xgp = ctx.enter_context(tc.tile_pool(name="xgp", bufs=len(_groups(KC0, True))))
        actA = ctx.enter_context(tc.tile_pool(name="actA", bufs=KC))
        actB = ctx.enter_context(tc.tile_pool(name="actB", bufs=KC))
        wp = ctx.enter_context(tc.tile_pool(name="wp", bufs=5))
        cons = ctx.enter_context(tc.tile_pool(name="cons", bufs=1))
        hwp = ctx.enter_context(tc.tile_pool(name="hwp", bufs=min(nslot, 10)))
        op = ctx.enter_context(tc.tile_pool(name="op", bufs=2))
        psp = ctx.enter_context(tc.tile_pool(name="psp", bufs=8, space="PSUM"))

        if repeat > 1:
            ctx.enter_context(tc.For_i(0, repeat, 1))

        bt = cons.tile([128, 3 * WC], F32, tag="bt")
        mt = cons.tile([128, nslot * HEAD_DIM], U8, tag="mt")
        consts_loaded = [False, False]

        xt = [None] * KC0

        def trunk_layer(src, wdram, nk, li, pool, tag, load_x=False):
            outs = [None] * WC
            for q in range(NQ):
                pss = [
                    psp.tile([128, BPC], F32, tag="ps", name=f"psL{li}q{q}w{w}")
                    for w in range(QW)
                ]
                for gi, (g0, glen) in enumerate(
                    _groups(nk, first_small=(li == 0 and q == 0))
                ):
                    # one DMA per k-chunk group: HWDGE descriptor generation
                    # is a serialized ~630ns/DMA shared resource, so fewer,
                    # larger transfers keep it off the critical path
                    wt = wp.tile([128, glen * WCOLS], BF16, tag="wp",
                                 name=f"wtL{li}q{q}g{gi}")
                    nc.sync.dma_start(
                        wt[:], wdram[q][:, g0 * WCOLS:(g0 + glen) * WCOLS]
                    )
                    if load_x and q == 0:
                        xg = xgp.tile([128, glen * BPC], BF16, tag="xg", name=f"xg{gi}")
                        nc.sync.dma_start(xg[:], xT[:, g0 * BPC:(g0 + glen) * BPC])
                        for k in range(g0, g0 + glen):
                            src[k] = xg[:, (k - g0) * BPC:(k - g0 + 1) * BPC]
                    for k in range(g0, g0 + glen):
                        o = (k - g0) * WCOLS
                        for w in range(QW):
                            nc.tensor.matmul(
                                pss[w][:],
                                wt[:, o + w * 128:o + (w + 1) * 128],
                                src[k],
                                start=(k == 0),
                                stop=(k == nk - 1),
                            )
                if li == 0 and q == 0 and not consts_loaded[0]:
                    # bias issued after L0-q0's operand loads (needed at the
                    # first relu, ~15us in), mask deferred to layer 2
                    nc.sync.dma_start(bt[:], bia)
                    consts_loaded[0] = True
                if li == 2 and q == 0 and not consts_loaded[1]:
                    nc.sync.dma_start(mt[:], msk)
                    consts_loaded[1] = True
                for w in range(QW):
                    wc_i = q * QW + w
                    h = pool.tile([128, BPC], BF16, tag=tag, name=f"h{li}_{wc_i}")
                    nc.scalar.activation(
                        h[:], pss[w][:], mybir.ActivationFunctionType.Relu,
                        bias=bt[:, li * WC + wc_i: li * WC + wc_i + 1],
                    )
                    outs[wc_i] = h[:]
            return outs

        h1 = trunk_layer(xt, w0, KC0, 0, actB, "actB", load_x=True)
        h2 = trunk_layer(h1, w1, KC, 1, actA, "actA")
        h3 = trunk_layer(h2, w2, KC, 2, actB, "actB")

        # Head, row-major: per window the stationary is the shared h3 column
        # block (one load per k for ALL slots); moving = head weights;
        # psum/out = [window rows (partitions), HEAD_DIM].
        sl0 = 0
        for win, sw in enumerate(struct):
            cols = slice(win * BLK, (win + 1) * BLK)
            hwts = []
            for s in range(sw):
                hw = hwp.tile([128, KC * HEAD_DIM], BF16, tag="hwp", name=f"hw{sl0 + s}")
                nc.sync.dma_start(hw[:], hws[sl0 + s])
                hwts.append(hw)
            pss = [
                psp.tile([128, HEAD_DIM], F32, tag="ps", name=f"psH{win}s{s}")
                for s in range(sw)
            ]
            for k in range(KC):
                for s in range(sw):
                    nc.tensor.matmul(
                        pss[s][:],
                        h3[k][:, cols],
                        hwts[s][:, k * HEAD_DIM:(k + 1) * HEAD_DIM],
                        start=(k == 0),
                        stop=(k == KC - 1),
                    )
            omw = op.tile([128, HEAD_DIM], F32, tag="op", name=f"om{win}")
            for s in range(sw):
                sl = sl0 + s
                if s == 0:
                    nc.vector.tensor_copy(omw[:], pss[s][:])
                else:
                    nc.vector.copy_predicated(
                        omw[:], mt[:, sl * HEAD_DIM:(sl + 1) * HEAD_DIM], pss[s][:]
                    )
            nc.sync.dma_start(out[win], omw[:])
            sl0 += sw

    nc.compile()
    return nc


def _pack_w(W, nk):
    # [NQ, 128, nk*QW*128]; [q, kp, k*512 + w*128+m] = W[k*128+kp, (q*QW+w)*128+m]
    return np.ascontiguousarray(
        W.reshape(nk, 128, NQ, QW * 128).transpose(2, 1, 0, 3)
        .reshape(NQ, 128, nk * QW * 128)
    )


def _pack_trunk(W0, W1, W2, b0, b1, b2):
    W0pad = np.zeros((KPAD, WIDTH), NP_BF16)
    W0pad[:KIN] = W0.astype(NP_BF16)
    w0p = _pack_w(W0pad, KC0)
    w1p = _pack_w(W1.astype(NP_BF16), KC)
    w2p = _pack_w(W2.astype(NP_BF16), KC)
    bias = np.zeros((128, 3 * WC), np.float32)
    for li, b in enumerate((b0, b1, b2)):
        bias[:, li * WC:(li + 1) * WC] = b.reshape(WC, 128).T
    return w0p, w1p, w2p, bias


def _exact_subset(rem, cap, maxn):
    """Subset of tasks from dict rem summing exactly to cap, at most maxn
    members, preferring fewest. Returns list of tasks or None."""
    dp = {0: (0, None, None)}
    for t, s in sorted(rem.items(), key=lambda kv: -kv[1]):
        for cur in sorted(dp.keys(), reverse=True):
            c = dp[cur][0]
            ns = cur + s
            if ns <= cap and c + 1 <= maxn and (ns not in dp or dp[ns][0] > c + 1):
                dp[ns] = (c + 1, t, cur)
    if cap not in dp:
        return None
    out, s = [], cap
    while s:
        _, t, prev = dp[s]
        out.append(t)
        s = prev
    return out


def _pack_blocks(sizes, rng, main_jitter):
    """Pack task row-counts into 128-row blocks, minimizing distinct tasks
    per block.  Each block: a "main" piece of the largest remaining task,
    then exact-subset fillers (fully consumed, no residues) or a single
    best-fit partial filler.  Returns list of blocks as [(task, n), ...]."""
    rem = {t: int(s) for t, s in sizes.items() if s > 0}
    nblocks = sum(rem.values()) // BLK
    blocks = []
    for _ in range(nblocks):
        blk, cap = [], BLK
        while cap > 0:
            if not blk:
                ts = sorted(rem, key=lambda t: -rem[t])
                j = int(rng.integers(0, min(main_jitter, len(ts)))) if len(ts) > 1 else 0
                pick = ts[j]
                take = min(rem[pick], cap)
            else:
                sub = _exact_subset(rem, cap, 2)
                if sub is not None:
                    for t in sub:
                        blk.append((t, rem[t]))
                        cap -= rem[t]
                        del rem[t]
                    continue
                geq = [t for t in rem if rem[t] > cap]
                good = [t for t in geq if rem[t] - cap >= 25]
                if good:
                    pick, take = min(good, key=lambda t: rem[t]), cap
                elif geq:
                    pick, take = min(geq, key=lambda t: rem[t]), cap
                else:
                    pick = max(rem, key=lambda t: rem[t])
                    take = min(rem[pick], cap)
            blk.append((pick, take))
            rem[pick] -= take
            if rem[pick] == 0:
                del rem[pick]
            cap -= take
        blocks.append(blk)
    assert not rem
    return blocks


def _assign(tid):
    """Choose the row permutation: pack tasks into 32 blocks of 128 rows,
    deal blocks to (core, window) with heavy blocks in window 0.  Returns
    (row_src, win_tasks, struct)."""
    sizes = np.bincount(tid, minlength=NUM_TASKS)
    nblocks = NCORES * NBLK
    best = None
    for seed in range(16):
        rng = np.random.default_rng(seed)
        blocks = _pack_blocks(dict(enumerate(sizes.tolist())), rng,
                              1 if seed == 0 else 5)
        idx = sorted(range(nblocks), key=lambda b: -len(blocks[b]))
        struct = tuple(len(blocks[idx[w * NCORES]]) for w in range(NBLK))
        key = (sum(struct), struct)
        if best is None or key < best[0]:
            best = (key, blocks, idx, struct)
        if sum(struct) <= NBLK * 2 + 1:
            break
    _, blocks, idx, struct = best

    rows_by_task = [np.nonzero(tid == t)[0] for t in range(NUM_TASKS)]
    pos = [0] * NUM_TASKS
    row_src = np.empty(BATCH, np.int64)
    win_tasks = []
    for c in range(NCORES):
        per = []
        for w in range(NBLK):
            blk = blocks[idx[w * NCORES + c]]
            lo = c * BPC + w * BLK
            o = 0
            tl = []
            for t, n in blk:
                row_src[lo + o: lo + o + n] = rows_by_task[t][pos[t]: pos[t] + n]
                pos[t] += n
                o += n
                tl.append(t)
            per.append(tl)
        win_tasks.append(per)
    return row_src, win_tasks, struct


def prepare(x, W0, b0, W1, b1, W2, b2, head_W, head_b):
    """Host-side sharding. Returns (in_maps, row_src, sorted_task_ids, struct).

    row_src[i] = original row index feeding padded-batch position i (core
    c, window w, offset j -> position c*BPC + w*BLK + j).
    """
    x = np.asarray(x, np.float32)
    W0 = np.asarray(W0, np.float32)
    W1 = np.asarray(W1, np.float32)
    W2 = np.asarray(W2, np.float32)
    b0 = np.asarray(b0, np.float32)
    b1 = np.asarray(b1, np.float32)
    b2 = np.asarray(b2, np.float32)
    head_W = np.asarray(head_W, np.float32)

    tid = np.argmax(x[:, -NUM_TASKS:], axis=1)
    row_src, win_tasks, struct = _assign(tid)
    t_s = tid[row_src]

    w0p, w1p, w2p, bias = _pack_trunk(W0, W1, W2, b0, b1, b2)
    head_Wb = head_W.astype(NP_BF16)
    # hw_pack[t, kp, kc*256 + j] = head_W[t, kc*128 + kp, j]
    hw_pack = np.ascontiguousarray(
        head_Wb.reshape(NUM_TASKS, KC, 128, HEAD_DIM)
        .transpose(0, 2, 1, 3)
        .reshape(NUM_TASKS, 128, KC * HEAD_DIM)
    )

    nslot = sum(struct)
    in_maps = []
    for c in range(NCORES):
        xs = x[row_src[c * BPC:(c + 1) * BPC]]
        xTp = np.zeros((KPAD, BPC), NP_BF16)
        xTp[:KIN] = xs.T.astype(NP_BF16)
        # flat [128, KC0*BPC]: [p, k*BPC + b] = x[b, k*128 + p]
        xTf = np.ascontiguousarray(
            xTp.reshape(KC0, 128, BPC).transpose(1, 0, 2).reshape(128, KC0 * BPC)
        )
        slot_tasks = []
        msk_c = np.zeros((128, nslot * HEAD_DIM), np.uint8)
        sl = 0
        for w in range(NBLK):
            tl = win_tasks[c][w]
            tl_p = tl + [tl[-1]] * (struct[w] - len(tl))
            lo = c * BPC + w * BLK
            ch = t_s[lo: lo + BLK]
            for s, t in enumerate(tl_p):
                slot_tasks.append(t)
                if 0 < s < len(tl):
                    # row mask: partition p of the window belongs to task t
                    msk_c[:, sl * HEAD_DIM:(sl + 1) * HEAD_DIM] = (
                        (ch == t)[:, None].astype(np.uint8)
                    )
                sl += 1
        hws_c = np.ascontiguousarray(hw_pack[np.asarray(slot_tasks)])
        in_maps.append({
            "xT": xTf,
            "w0p": w0p, "w1p": w1p, "w2p": w2p, "bias": bias,
            "hws": hws_c, "msk": msk_c,
        })
    return in_maps, row_src, t_s, struct


def _assemble(results, row_src, t_s, head_b):
    head_b = np.asarray(head_b, np.float32)
    outs = []
    for c in range(NCORES):
        oT = results[c]["outT"]                       # [NBLK, 128, HEAD_DIM]
        outs.append(oT.reshape(BPC, HEAD_DIM))        # row-major window rows
    out_s = np.concatenate(outs, axis=0) + head_b[t_s]
    out = np.empty_like(out_s)
    out[row_src] = out_s
    return out.astype(np.float32)


def kernel(x, W0, b0, W1, b1, W2, b2, head_W, head_b):
    in_maps, row_src, t_s, struct = prepare(x, W0, b0, W1, b1, W2, b2, head_W, head_b)
    nc = _PROG_CACHE.get(struct)
    if nc is None:
        nc = _build(struct)
        _PROG_CACHE[struct] = nc
    res = bass_utils.run_bass_kernel_spmd(nc, in_maps, core_ids=list(range(NCORES)))
    return _assemble(res.results, row_src, t_s, head_b)
